# revision 69
# baseline (speedup 1.0000x reference)
"""Masked multi-head self-attention for Trainium2, SPMD over 8 NeuronCores.

Sharding: core c handles batch c//2, query-half c%2 (1024 of 2048 query rows).
The same Bass program runs on every core; odd cores get their inputs rotated
along the key axis so that "my" query rows are always tokens [0, 1024)
(attention sums are invariant to a consistent permutation of the key axis).

Host supplies x^T (features-major q), (1-mask)^T, and all weights in fp16,
so the device does no transposes. Per-core dataflow:
  Q^T/K^T (head-pair packed fp16) and V (token-major fp16, with a ones
        column for the softmax denominator) via PE projections from x^T
  per head h, key-tile gt: S^T = K @ Q^T into fp32 PSUM (fp16 operands)
  U = exp(0.125 * S^T - 8) on ACT (PSUM -> SBUF fp16), masked by
        (1-mask)^T via one fp16 multiply (DVE or GPSIMD, balanced; the
        GPSIMD ones are consumed last in the hv accumulation for slack)
  heads[q,dk] + denominator column via PE with U^T chunks stationary and
        [V | 1] moving -> PSUM [128q, 4, 65] quads per (h, q-chunk)
  normalize out of PSUM with a per-partition reciprocal + tensor_scalar
        (q is the partition axis, so no cross-partition broadcast), fp16
  PE-transpose the normalized chunk into head-pair-packed heads^T layout
  output projection: 4 pair-packed accumulation steps per query tile.

Scheduling: the exp stream on ACT is the critical resource (~133us busy),
so everything else is spread to keep it fed: the S^T PSUM ring is 3 deep
(6 banks); hv/transpose/projection scratch shares a 2-bank ring; each
head's hv+normalize work and the next head-pair's projections are
interleaved into the NEXT head's S-loop in evenly spread slots (by which
point all of the head's U tiles exist, so the quads never stall); head 7
fuses the output projection into its normalize quads to shorten the tail.
(Note: emitting the hv accumulation gt-major instead of qc-major looks
3us faster in TimelineSim but corrupts results on hardware — interleaved
PSUM accumulation groups are not safe there. A two-pass variant that
closes each chunk's group after 14 gts and reopens with start=False for
the last two ALSO corrupts on hardware, so no form of split or
interleaved accumulation within a shared PSUM bank should be used, even
though both pass CoreSim and TimelineSim.)
"""

import sys

sys.path.insert(0, "/opt/trn_rl_repo")

import numpy as np

import concourse.bass as bass  # noqa: F401
import concourse.tile as tile
from concourse import bacc, mybir
from concourse.bass_utils import run_bass_kernel_spmd
from concourse.masks import make_identity

F32 = mybir.dt.float32
F16 = mybir.dt.float16
EXP = mybir.ActivationFunctionType.Exp
MUL = mybir.AluOpType.mult

B, N, D, H, DK = 4, 2048, 512, 8, 64
# Schraudolph bit-exp (int32 bitcast as f32): exp(0.125*s - 8) ~=
# bitcast(int32(ALPHA*s + BETA)); the mask rides in BETA - 240*ALPHA*m.
LOG2E = 1.4426950408889634
ALPHA = 0.125 * LOG2E * (1 << 23)
SCH_C = 366000.0
BETA = (127.0 - 8.0 * LOG2E) * (1 << 23) - SCH_C
MBGTS = ()                  # key tiles computed via bit-exp on DVE (off:
                            # f32r stationary can't mix with fp16 moving)
NQ = N // 2          # query rows per core
NORM = 1.0 / 8.0     # 1/sqrt(DK)
NFC = D // 128       # feature chunks (4)
NHP = H // 2         # head pairs (4)
NGT = N // 128       # key tiles (16)
NQC = NQ // 128      # query chunks per core (8)
NCORES = 8

_CACHE = {}


def _build():
    if "nc" in _CACHE:
        return _CACHE["nc"]
    nc = bacc.Bacc("TRN2", target_bir_lowering=False, debug=False,
                   num_devices=NCORES)
    xqt = nc.dram_tensor("xqt", [D, N], F16, kind="ExternalInput")
    nmtd = nc.dram_tensor("nmt", [N, NQ], F16, kind="ExternalInput")
    wq = nc.dram_tensor("wq", [D, D], F16, kind="ExternalInput")
    wk = nc.dram_tensor("wk", [D, D], F16, kind="ExternalInput")
    wv = nc.dram_tensor("wv", [D, D], F16, kind="ExternalInput")
    wo = nc.dram_tensor("wo", [128, NHP * D], F16, kind="ExternalInput")
    # head-pair-0 chunks of wq|wk, host-packed partition-major so the lead
    # DMA is one transfer with contiguous 1KB runs (no sub-512B penalty)
    wqk0d = nc.dram_tensor("wqk0", [128, 2 * NFC * 128], F16,
                           kind="ExternalInput")
    if MBGTS:
        mbd = nc.dram_tensor("mb", [len(MBGTS) * 128, NQ], F32,
                             kind="ExternalInput")
    out = nc.dram_tensor("out", [NQ, D], F32, kind="ExternalOutput")

    with tile.TileContext(nc) as tc:
        with tc.tile_pool(name="persist", bufs=1) as P, \
             tc.tile_pool(name="ub", bufs=2) as UB, \
             tc.tile_pool(name="rn", bufs=6) as RN, \
             tc.tile_pool(name="spsum", bufs=3, space="PSUM") as SP, \
             tc.tile_pool(name="hpsum", bufs=2, space="PSUM") as HP:
            XT = tc.alloc_tile_pool(name="xw", bufs=1)
            kt = P.tile([128, NHP, N], F16)     # K^T, two heads per tile
            qt_ = P.tile([128, NHP, NQ], F16)   # Q^T, two heads per tile
            v1 = P.tile([128, NGT, H, DK + 1], F16)  # V | ones, token-major
            nmt = P.tile([128, NGT, NQ], F16)        # (1-mask)^T
            htn = P.tile([128, NHP, NQ], F16)        # normalized heads^T
            wob = P.tile([128, NHP * D], F16)
            ident = P.tile([128, 128], F16)
            nbias = P.tile([128, 1], F32)
            nc.vector.memset(nbias[:], -8.0)

            if MBGTS:
                mb = P.tile([128, len(MBGTS), NQ], F32)
                nc.sync.dma_start(out=mb[:],
                                  in_=mbd.rearrange("(g p) q -> p g q", p=128))
            xt = XT.tile([128, NFC, N], F16)
            wqb = XT.tile([128, NFC, D], F16)
            wkb = XT.tile([128, NFC, D], F16)
            wvb = XT.tile([128, NFC, D], F16)

            wqk0b = XT.tile([128, 2, NFC, 128], F16)
            wqv = wq.rearrange("(fc p) d -> p fc d", p=128)
            wkv = wk.rearrange("(fc p) d -> p fc d", p=128)
            xv = xqt.rearrange("(fc p) n -> p fc n", p=128)
            nc.sync.dma_start(out=wqk0b[:],
                              in_=wqk0d.rearrange("p (j fc c) -> p j fc c",
                                                  j=2, fc=NFC))
            nc.sync.dma_start(out=xt[:, :, 0:512], in_=xv[:, :, 0:512])
            nc.sync.dma_start(out=xt[:, :, 512:NQ], in_=xv[:, :, 512:NQ])
            nc.sync.dma_start(out=wqb[:, :, 128:D], in_=wqv[:, :, 128:D])
            nc.sync.dma_start(out=wkb[:, :, 128:D], in_=wkv[:, :, 128:D])
            nc.sync.dma_start(out=xt[:, :, NQ:N],
                              in_=xqt.rearrange("(fc p) n -> p fc n",
                                                p=128)[:, :, NQ:N])
            nc.sync.dma_start(out=wvb[:],
                              in_=wv.rearrange("(fc p) d -> p fc d", p=128))
            nmtv = nmtd.rearrange("(gc p) q -> p gc q", p=128)
            for gc4 in range(4):
                nc.sync.dma_start(out=nmt[:, gc4 * 4:(gc4 + 1) * 4, :],
                                  in_=nmtv[:, gc4 * 4:(gc4 + 1) * 4, :])
            nc.sync.dma_start(out=wob[:], in_=wo[:, :])

            nc.vector.memset(v1[:, :, :, DK:DK + 1], 1.0)
            make_identity(nc, ident)
            # warm the ACT exp table while the input DMAs are in flight
            nc.scalar.activation(nbias[:], nbias[:], EXP, bias=nbias[:],
                                 scale=0.0)
            nc.vector.memset(nbias[:], -8.0)
            # keep the PE continuously busy through the DMA wait so it is
            # at full clock (p-state ramp needs ~3us) when the first real
            # projection issues
            wps = HP.tile([128, 128], F32, tag="hvtp", name="wps")
            for _ in range(18):
                nc.tensor.matmul(wps[:], ident[:], ident[:],
                                 start=True, stop=True)

            def _proj_group(wsel, dst, hp, ttg):
                ps = HP.tile([128, 512], F32, tag="hvtp", name="psp")
                for fc in range(NFC):
                    nc.tensor.matmul(
                        ps[:], wsel(fc),
                        xt[:, fc, ttg * 512:(ttg + 1) * 512],
                        start=(fc == 0), stop=(fc == NFC - 1))
                nc.vector.tensor_copy(
                    dst[:, hp, ttg * 512:(ttg + 1) * 512], ps[:])

            def proj_qk_groups(hp):
                if hp == 0:
                    wq_s = lambda fc: wqk0b[:, 0, fc, :]
                    wk_s = lambda fc: wqk0b[:, 1, fc, :]
                else:
                    wq_s = lambda fc: wqb[:, fc, hp * 128:(hp + 1) * 128]
                    wk_s = lambda fc: wkb[:, fc, hp * 128:(hp + 1) * 128]
                return ([lambda ttg=ttg: _proj_group(wq_s, qt_, hp, ttg)
                         for ttg in range(2)] +
                        [lambda ttg=ttg: _proj_group(wk_s, kt, hp, ttg)
                         for ttg in range(4)])

            def proj_qk(hp):
                for g in proj_qk_groups(hp):
                    g()

            def _proj_v_group(gt, half):
                ps = HP.tile([128, 256], F32, tag="hvtp", name="psv")
                for fc in range(NFC):
                    nc.tensor.matmul(
                        ps[:],
                        xt[:, fc, gt * 128:(gt + 1) * 128],
                        wvb[:, fc, half * 256:(half + 1) * 256],
                        start=(fc == 0), stop=(fc == NFC - 1))
                nc.vector.tensor_copy(
                    v1[:, gt, half * 4:(half + 1) * 4, 0:DK],
                    ps.rearrange("p (h v) -> p h v", h=4))

            def proj_v_groups(half):
                return [lambda gt=gt: _proj_v_group(gt, half)
                        for gt in range(NGT)]

            # Pool (gpsimd) takes some mask multiplies; they are consumed
            # last in the hv accumulation so their extra latency has slack.
            # The final heads keep everything on DVE so the tail is short.
            def poolgt(h):
                if h >= 7:
                    return ()
                if h == 6:
                    return (1, 7)
                return (1, 4, 7, 10, 13)

            def attn_head(h, interleave=(), post_quad=None, defer_hv=False):
                hp, i = divmod(h, 2)
                pgt = poolgt(h)
                gtorder = [g for g in range(NGT) if g not in pgt] + list(pgt)
                post_sloop = []
                items = list(interleave)
                if len(items) > 14:
                    post_sloop = items[14:]
                    items = items[:14]
                inter = {}
                n = len(items)
                for j, g in enumerate(items):
                    slot = 1 + round(j * 14 / (n - 1)) if n > 1 else 1
                    inter.setdefault(min(slot, 15), []).append(g)

                u = UB.tile([128, NGT - len(MBGTS), NQ], F16, tag="u",
                            name="u")
                u32 = UB.tile([128, max(1, len(MBGTS)), NQ], mybir.dt.int32,
                              tag="u32", name="u32") if MBGTS else None
                ugx = {}
                for gt in range(NGT):
                    if gt not in MBGTS:
                        ugx[gt] = len(ugx)
                for gt in range(NGT):
                    for g in inter.get(gt, ()):
                        g()
                    s = SP.tile([128, NQ], F32, tag="s2", name="s")
                    for qg in range(2):
                        nc.tensor.matmul(
                            s[:, qg * 512:(qg + 1) * 512],
                            kt[i * 64:(i + 1) * 64, hp,
                               gt * 128:(gt + 1) * 128],
                            qt_[i * 64:(i + 1) * 64, hp,
                                qg * 512:(qg + 1) * 512],
                            start=True, stop=True)
                    if gt in MBGTS:
                        mi = MBGTS.index(gt)
                        nc.vector.scalar_tensor_tensor(
                            u32[:, mi, :], s[:], ALPHA, mb[:, mi, :],
                            MUL, mybir.AluOpType.add)
                    elif h == H - 1 and gt == NGT - 1:
                        # final tile of the final head: exp+mask in halves so
                        # the first hv chains (query chunks 0-3) start before
                        # the second half finishes
                        for qg in range(2):
                            sl = slice(qg * 512, (qg + 1) * 512)
                            nc.scalar.activation(u[:, ugx[gt], sl], s[:, sl],
                                                 EXP, bias=nbias[:],
                                                 scale=NORM)
                            nc.vector.tensor_mul(u[:, ugx[gt], sl],
                                                 u[:, ugx[gt], sl],
                                                 nmt[:, gt, sl])
                    else:
                        nc.scalar.activation(u[:, ugx[gt], :], s[:], EXP,
                                             bias=nbias[:], scale=NORM)
                        eng = nc.gpsimd if gt in pgt else nc.vector
                        eng.tensor_mul(u[:, ugx[gt], :], u[:, ugx[gt], :],
                                       nmt[:, gt, :])

                for g in post_sloop:
                    g()

                # 4 query-chunks per PSUM bank; hv accumulation for the next
                # quad is emitted before the (DVE-dependent) normalize +
                # transpose of the previous one, so the in-order PE queue
                # never blocks on the normalize chain.
                def hv_quad(q0):
                    hps = HP.tile([128, 4, DK + 1], F32, tag="hvtp",
                                  name="hps")
                    u32f = u32.bitcast(mybir.dt.float32r) if MBGTS else None
                    for dq in range(4):
                        qc = q0 + dq
                        for j, gt in enumerate(gtorder):
                            if gt in MBGTS:
                                st = u32f[:, MBGTS.index(gt),
                                          qc * 128:(qc + 1) * 128]
                            else:
                                st = u[:, ugx[gt],
                                       qc * 128:(qc + 1) * 128]
                            nc.tensor.matmul(
                                hps[:, dq, :], st, v1[:, gt, h, :],
                                start=(j == 0), stop=(j == NGT - 1))
                    return hps

                def norm_quad(q0, hps):
                    tp = HP.tile([128, 4, 128], F16, tag="hvtp", name="tp")
                    hns = []
                    for dq in range(4):
                        rinv = RN.tile([128, 1], F32, tag="rinv", name="rinv")
                        nc.vector.reciprocal_approx_fast(
                            rinv[:], hps[:, dq, DK:DK + 1])
                        hnorm = RN.tile([128, DK], F16, tag="hnorm",
                                        name="hnorm")
                        nc.vector.tensor_scalar(hnorm[:], hps[:, dq, 0:DK],
                                                rinv[:], None, MUL)
                        hns.append(hnorm)
                    for dq in range(4):
                        nc.tensor.transpose(tp[i * 64:(i + 1) * 64, dq, :],
                                            hns[dq][:], ident[:],
                                            tile_position=(0, i * 64))
                    for dq in range(4):
                        qc = q0 + dq
                        nc.vector.tensor_copy(
                            htn[i * 64:(i + 1) * 64, hp,
                                qc * 128:(qc + 1) * 128],
                            tp[i * 64:(i + 1) * 64, dq, :])

                def chunk0():
                    hvs[0] = hv_quad(0)
                def chunk1():
                    hvs[1] = hv_quad(4)
                def chunk2():
                    norm_quad(0, hvs[0])
                    if post_quad:
                        post_quad(0)
                def chunk3():
                    norm_quad(4, hvs[1])
                    if post_quad:
                        post_quad(1)
                hvs = [None, None]
                chunks = [chunk0, chunk1, chunk2, chunk3]
                if defer_hv:
                    return chunks
                for c in chunks:
                    c()

            pq0 = proj_qk_groups(0)
            for g in pq0[:4]:
                g()
            vb = proj_v_groups(1)
            prev = attn_head(0, interleave=pq0[4:] + proj_v_groups(0),
                             defer_hv=True)
            prev = attn_head(1, interleave=proj_qk_groups(1) + prev,
                             defer_hv=True)
            prev = attn_head(2, interleave=vb[:8] + prev, defer_hv=True)
            prev = attn_head(3, interleave=proj_qk_groups(2) + prev,
                             defer_hv=True)
            prev = attn_head(4, interleave=vb[8:] + prev, defer_hv=True)
            prev = attn_head(5, interleave=proj_qk_groups(3) + prev,
                             defer_hv=True)
            prev = attn_head(6, interleave=prev, defer_hv=True)
            with tc.tile_pool(name="ob", bufs=6) as OB:
                def out_proj(qts):
                    for qt in qts:
                        po = SP.tile([128, 512], F32, tag="s2", name="po")
                        for hp in range(NHP):
                            nc.tensor.matmul(
                                po[:],
                                htn[:, hp, qt * 128:(qt + 1) * 128],
                                wob[:, hp * 512:(hp + 1) * 512],
                                start=(hp == 0), stop=(hp == NHP - 1))
                        ob = OB.tile([128, 512], F32, tag="ob", name="ob")
                        nc.scalar.copy(ob[:], po[:])
                        nc.sync.dma_start(
                            out=out[qt * 128:(qt + 1) * 128, :], in_=ob[:])

                attn_head(7, interleave=prev,
                          post_quad=lambda half: out_proj(
                              range(half * 4, half * 4 + 4)))
            XT.release()


    nc.compile()
    _CACHE["nc"] = nc
    return nc


def kernel(q, mask, W_query, W_key, W_val, W_out):
    q = np.asarray(q, dtype=np.float32)
    mask = np.asarray(mask, dtype=np.int32)
    # [f, hp*128 + i*64 + dk] for the q/k projections (head-pair packed),
    # [f, h*64 + dv] for v, [i*64 + dk, hp*512 + e] for the output projection.
    wq_r = np.ascontiguousarray(np.transpose(
        np.asarray(W_query, np.float32), (1, 0, 2)).reshape(D, D)).astype(np.float16)
    wk_r = np.ascontiguousarray(np.transpose(
        np.asarray(W_key, np.float32), (1, 0, 2)).reshape(D, D)).astype(np.float16)
    wv_r = np.ascontiguousarray(np.transpose(
        np.asarray(W_val, np.float32), (1, 0, 2)).reshape(D, D)).astype(np.float16)
    wo_r = np.ascontiguousarray(
        np.asarray(W_out, np.float32).reshape(NHP, 2, DK, D)
        .transpose(1, 2, 0, 3).reshape(128, NHP * D)).astype(np.float16)

    nc = _build()
    in_maps = []
    for c in range(NCORES):
        b, qh = c // 2, c % 2
        xqt_c = q[b].T                                      # (D, N)
        nmt_c = 1.0 - mask[b, qh * NQ:(qh + 1) * NQ, :].T   # (N, NQ)
        if qh:
            # rotate the key axis so this core's queries are tokens [0, NQ)
            xqt_c = np.roll(xqt_c, -NQ, axis=1)
            nmt_c = np.roll(nmt_c, -NQ, axis=0)
        wqk0 = np.concatenate(
            [wr[:, 0:128].reshape(NFC, 128, 128).transpose(1, 0, 2)
             .reshape(128, NFC * 128) for wr in (wq_r, wk_r)],
            axis=1)
        imap = {
            "wqk0": np.ascontiguousarray(wqk0),
            "xqt": np.ascontiguousarray(xqt_c.astype(np.float16)),
            "nmt": np.ascontiguousarray(nmt_c.astype(np.float16)),
            "wq": wq_r, "wk": wk_r, "wv": wv_r, "wo": wo_r,
        }
        if MBGTS:
            mrows = np.concatenate([1.0 - nmt_c[g * 128:(g + 1) * 128, :]
                                    for g in MBGTS], axis=0)
            imap["mb"] = np.ascontiguousarray(
                np.float32(BETA)
                - np.float32(240.0 * ALPHA) * mrows.astype(np.float32))
        in_maps.append(imap)
    res = run_bass_kernel_spmd(nc, in_maps, core_ids=list(range(NCORES)))
    output = np.empty((B, N, D), np.float32)
    for c in range(NCORES):
        b, qh = c // 2, c % 2
        output[b, qh * NQ:(qh + 1) * NQ, :] = res.results[c]["out"]
    return output


# revision 72
# speedup vs baseline: 1.0005x; 1.0005x over previous
"""Masked multi-head self-attention for Trainium2, SPMD over 8 NeuronCores.

Sharding: core c handles batch c//2, query-half c%2 (1024 of 2048 query rows).
The same Bass program runs on every core; odd cores get their inputs rotated
along the key axis so that "my" query rows are always tokens [0, 1024)
(attention sums are invariant to a consistent permutation of the key axis).

Host supplies x^T (features-major q), (1-mask)^T, and all weights in fp16,
so the device does no transposes. Per-core dataflow:
  Q^T/K^T (head-pair packed fp16) and V (token-major fp16, with a ones
        column for the softmax denominator) via PE projections from x^T
  per head h, key-tile gt: S^T = K @ Q^T into fp32 PSUM (fp16 operands)
  U = exp(0.125 * S^T - 8) on ACT (PSUM -> SBUF fp16), masked by
        (1-mask)^T via one fp16 multiply (DVE or GPSIMD, balanced; the
        GPSIMD ones are consumed last in the hv accumulation for slack)
  heads[q,dk] + denominator column via PE with U^T chunks stationary and
        [V | 1] moving -> PSUM [128q, 4, 65] quads per (h, q-chunk)
  normalize out of PSUM with a per-partition reciprocal + tensor_scalar
        (q is the partition axis, so no cross-partition broadcast), fp16
  PE-transpose the normalized chunk into head-pair-packed heads^T layout
  output projection: 4 pair-packed accumulation steps per query tile.

Scheduling: the exp stream on ACT is the critical resource (~133us busy),
so everything else is spread to keep it fed: the S^T PSUM ring is 3 deep
(6 banks); hv/transpose/projection scratch shares a 2-bank ring; each
head's hv+normalize work and the next head-pair's projections are
interleaved into the NEXT head's S-loop in evenly spread slots (by which
point all of the head's U tiles exist, so the quads never stall); head 7
fuses the output projection into its normalize quads to shorten the tail.
(Note: emitting the hv accumulation gt-major instead of qc-major looks
3us faster in TimelineSim but corrupts results on hardware — interleaved
PSUM accumulation groups are not safe there. A two-pass variant that
closes each chunk's group after 14 gts and reopens with start=False for
the last two ALSO corrupts on hardware, so no form of split or
interleaved accumulation within a shared PSUM bank should be used, even
though both pass CoreSim and TimelineSim.)
"""

import sys

sys.path.insert(0, "/opt/trn_rl_repo")

import numpy as np

import concourse.bass as bass  # noqa: F401
import concourse.tile as tile
from concourse import bacc, mybir
from concourse.bass_utils import run_bass_kernel_spmd
from concourse.masks import make_identity

F32 = mybir.dt.float32
F16 = mybir.dt.float16
EXP = mybir.ActivationFunctionType.Exp
MUL = mybir.AluOpType.mult

B, N, D, H, DK = 4, 2048, 512, 8, 64
# Schraudolph bit-exp (int32 bitcast as f32): exp(0.125*s - 8) ~=
# bitcast(int32(ALPHA*s + BETA)); the mask rides in BETA - 240*ALPHA*m.
LOG2E = 1.4426950408889634
ALPHA = 0.125 * LOG2E * (1 << 23)
SCH_C = 366000.0
BETA = (127.0 - 8.0 * LOG2E) * (1 << 23) - SCH_C
MBGTS = ()                  # key tiles computed via bit-exp on DVE (off:
                            # f32r stationary can't mix with fp16 moving)
NQ = N // 2          # query rows per core
NORM = 1.0 / 8.0     # 1/sqrt(DK)
NFC = D // 128       # feature chunks (4)
NHP = H // 2         # head pairs (4)
NGT = N // 128       # key tiles (16)
NQC = NQ // 128      # query chunks per core (8)
NCORES = 8

_CACHE = {}


def _build():
    if "nc" in _CACHE:
        return _CACHE["nc"]
    nc = bacc.Bacc("TRN2", target_bir_lowering=False, debug=False,
                   num_devices=NCORES)
    xqt = nc.dram_tensor("xqt", [D, N], F16, kind="ExternalInput")
    nmtd = nc.dram_tensor("nmt", [N, NQ], F16, kind="ExternalInput")
    wq = nc.dram_tensor("wq", [D, D], F16, kind="ExternalInput")
    wk = nc.dram_tensor("wk", [D, D], F16, kind="ExternalInput")
    wv = nc.dram_tensor("wv", [D, D], F16, kind="ExternalInput")
    wo = nc.dram_tensor("wo", [128, NHP * D], F16, kind="ExternalInput")
    # head-pair-0 chunks of wq|wk, host-packed partition-major so the lead
    # DMA is one transfer with contiguous 1KB runs (no sub-512B penalty)
    wqk0d = nc.dram_tensor("wqk0", [128, 2 * NFC * 128], F16,
                           kind="ExternalInput")
    if MBGTS:
        mbd = nc.dram_tensor("mb", [len(MBGTS) * 128, NQ], F32,
                             kind="ExternalInput")
    out = nc.dram_tensor("out", [NQ, D], F32, kind="ExternalOutput")

    with tile.TileContext(nc) as tc:
        with tc.tile_pool(name="persist", bufs=1) as P, \
             tc.tile_pool(name="ub", bufs=2) as UB, \
             tc.tile_pool(name="rn", bufs=6) as RN, \
             tc.tile_pool(name="spsum", bufs=3, space="PSUM") as SP, \
             tc.tile_pool(name="hpsum", bufs=2, space="PSUM") as HP:
            XT = tc.alloc_tile_pool(name="xw", bufs=1)
            kt = P.tile([128, NHP, N], F16)     # K^T, two heads per tile
            qt_ = P.tile([128, NHP, NQ], F16)   # Q^T, two heads per tile
            v1 = P.tile([128, NGT, H, DK + 1], F16)  # V | ones, token-major
            nmt = P.tile([128, NGT, NQ], F16)        # (1-mask)^T
            htn = P.tile([128, NHP, NQ], F16)        # normalized heads^T
            wob = P.tile([128, NHP * D], F16)
            ident = P.tile([128, 128], F16)
            nbias = P.tile([128, 1], F32)
            nc.vector.memset(nbias[:], -8.0)

            if MBGTS:
                mb = P.tile([128, len(MBGTS), NQ], F32)
                nc.sync.dma_start(out=mb[:],
                                  in_=mbd.rearrange("(g p) q -> p g q", p=128))
            xt = XT.tile([128, NFC, N], F16)
            wqb = XT.tile([128, NFC, D], F16)
            wkb = XT.tile([128, NFC, D], F16)
            wvb = XT.tile([128, NFC, D], F16)

            wqk0b = XT.tile([128, 2, NFC, 128], F16)
            wqv = wq.rearrange("(fc p) d -> p fc d", p=128)
            wkv = wk.rearrange("(fc p) d -> p fc d", p=128)
            xv = xqt.rearrange("(fc p) n -> p fc n", p=128)
            nc.sync.dma_start(out=wqk0b[:],
                              in_=wqk0d.rearrange("p (j fc c) -> p j fc c",
                                                  j=2, fc=NFC))
            nc.sync.dma_start(out=xt[:, :, 0:512], in_=xv[:, :, 0:512])
            nc.sync.dma_start(out=xt[:, :, 512:NQ], in_=xv[:, :, 512:NQ])
            nc.sync.dma_start(out=wqb[:, :, 128:D], in_=wqv[:, :, 128:D])
            nc.sync.dma_start(out=wkb[:, :, 128:D], in_=wkv[:, :, 128:D])
            nc.sync.dma_start(out=xt[:, :, NQ:N],
                              in_=xqt.rearrange("(fc p) n -> p fc n",
                                                p=128)[:, :, NQ:N])
            nc.sync.dma_start(out=wvb[:],
                              in_=wv.rearrange("(fc p) d -> p fc d", p=128))
            nmtv = nmtd.rearrange("(gc p) q -> p gc q", p=128)
            for gc4 in range(4):
                nc.sync.dma_start(out=nmt[:, gc4 * 4:(gc4 + 1) * 4, :],
                                  in_=nmtv[:, gc4 * 4:(gc4 + 1) * 4, :])
            nc.sync.dma_start(out=wob[:], in_=wo[:, :])

            nc.vector.memset(v1[:, :, :, DK:DK + 1], 1.0)
            make_identity(nc, ident)
            # warm the ACT exp table while the input DMAs are in flight
            nc.scalar.activation(nbias[:], nbias[:], EXP, bias=nbias[:],
                                 scale=0.0)
            nc.vector.memset(nbias[:], -8.0)
            # keep the PE continuously busy through the DMA wait so it is
            # at full clock (p-state ramp needs ~3us) when the first real
            # projection issues
            wps = HP.tile([128, 128], F32, tag="hvtp", name="wps")
            for _ in range(18):
                nc.tensor.matmul(wps[:], ident[:], ident[:],
                                 start=True, stop=True)

            def _proj_group(wsel, dst, hp, ttg):
                ps = HP.tile([128, 512], F32, tag="hvtp", name="psp")
                for fc in range(NFC):
                    nc.tensor.matmul(
                        ps[:], wsel(fc),
                        xt[:, fc, ttg * 512:(ttg + 1) * 512],
                        start=(fc == 0), stop=(fc == NFC - 1))
                nc.vector.tensor_copy(
                    dst[:, hp, ttg * 512:(ttg + 1) * 512], ps[:])

            def proj_qk_groups(hp):
                if hp == 0:
                    wq_s = lambda fc: wqk0b[:, 0, fc, :]
                    wk_s = lambda fc: wqk0b[:, 1, fc, :]
                else:
                    wq_s = lambda fc: wqb[:, fc, hp * 128:(hp + 1) * 128]
                    wk_s = lambda fc: wkb[:, fc, hp * 128:(hp + 1) * 128]
                return ([lambda ttg=ttg: _proj_group(wq_s, qt_, hp, ttg)
                         for ttg in range(2)] +
                        [lambda ttg=ttg: _proj_group(wk_s, kt, hp, ttg)
                         for ttg in range(4)])

            def proj_qk(hp):
                for g in proj_qk_groups(hp):
                    g()

            def _proj_v_group(gt, half):
                ps = HP.tile([128, 256], F32, tag="hvtp", name="psv")
                for fc in range(NFC):
                    nc.tensor.matmul(
                        ps[:],
                        xt[:, fc, gt * 128:(gt + 1) * 128],
                        wvb[:, fc, half * 256:(half + 1) * 256],
                        start=(fc == 0), stop=(fc == NFC - 1))
                nc.vector.tensor_copy(
                    v1[:, gt, half * 4:(half + 1) * 4, 0:DK],
                    ps.rearrange("p (h v) -> p h v", h=4))

            def proj_v_groups(half):
                return [lambda gt=gt: _proj_v_group(gt, half)
                        for gt in range(NGT)]

            # Pool (gpsimd) takes some mask multiplies; they are consumed
            # last in the hv accumulation so their extra latency has slack.
            # The final heads keep everything on DVE so the tail is short.
            def poolgt(h):
                if h >= 7:
                    return ()
                if h == 6:
                    return (1, 7)
                return (1, 4, 7, 10, 13)

            def attn_head(h, interleave=(), post_quad=None, defer_hv=False):
                hp, i = divmod(h, 2)
                pgt = poolgt(h)
                gtorder = [g for g in range(NGT) if g not in pgt] + list(pgt)
                post_sloop = []
                items = list(interleave)
                if len(items) > 14:
                    post_sloop = items[14:]
                    items = items[:14]
                inter = {}
                n = len(items)
                for j, g in enumerate(items):
                    slot = 1 + round(j * 14 / (n - 1)) if n > 1 else 1
                    inter.setdefault(min(slot, 15), []).append(g)

                u = UB.tile([128, NGT - len(MBGTS), NQ], F16, tag="u",
                            name="u")
                u32 = UB.tile([128, max(1, len(MBGTS)), NQ], mybir.dt.int32,
                              tag="u32", name="u32") if MBGTS else None
                ugx = {}
                for gt in range(NGT):
                    if gt not in MBGTS:
                        ugx[gt] = len(ugx)
                for gt in range(NGT):
                    for g in inter.get(gt, ()):
                        g()
                    s = SP.tile([128, NQ], F32, tag="s2", name="s")
                    for qg in range(2):
                        nc.tensor.matmul(
                            s[:, qg * 512:(qg + 1) * 512],
                            kt[i * 64:(i + 1) * 64, hp,
                               gt * 128:(gt + 1) * 128],
                            qt_[i * 64:(i + 1) * 64, hp,
                                qg * 512:(qg + 1) * 512],
                            start=True, stop=True)
                    if gt in MBGTS:
                        mi = MBGTS.index(gt)
                        nc.vector.scalar_tensor_tensor(
                            u32[:, mi, :], s[:], ALPHA, mb[:, mi, :],
                            MUL, mybir.AluOpType.add)
                    elif h == H - 1 and gt == NGT - 1:
                        # final tile of the final head: exp+mask in halves so
                        # the first hv chains (query chunks 0-3) start before
                        # the second half finishes
                        for qg in range(2):
                            sl = slice(qg * 512, (qg + 1) * 512)
                            nc.scalar.activation(u[:, ugx[gt], sl], s[:, sl],
                                                 EXP, bias=nbias[:],
                                                 scale=NORM)
                            nc.vector.tensor_mul(u[:, ugx[gt], sl],
                                                 u[:, ugx[gt], sl],
                                                 nmt[:, gt, sl])
                    else:
                        nc.scalar.activation(u[:, ugx[gt], :], s[:], EXP,
                                             bias=nbias[:], scale=NORM)
                        eng = nc.gpsimd if gt in pgt else nc.vector
                        eng.tensor_mul(u[:, ugx[gt], :], u[:, ugx[gt], :],
                                       nmt[:, gt, :])

                for g in post_sloop:
                    g()

                # 4 query-chunks per PSUM bank; hv accumulation for the next
                # quad is emitted before the (DVE-dependent) normalize +
                # transpose of the previous one, so the in-order PE queue
                # never blocks on the normalize chain.
                def hv_quad(q0):
                    hps = HP.tile([128, 4, DK + 1], F32, tag="hvtp",
                                  name="hps")
                    u32f = u32.bitcast(mybir.dt.float32r) if MBGTS else None
                    for dq in range(4):
                        qc = q0 + dq
                        for j, gt in enumerate(gtorder):
                            if gt in MBGTS:
                                st = u32f[:, MBGTS.index(gt),
                                          qc * 128:(qc + 1) * 128]
                            else:
                                st = u[:, ugx[gt],
                                       qc * 128:(qc + 1) * 128]
                            nc.tensor.matmul(
                                hps[:, dq, :], st, v1[:, gt, h, :],
                                start=(j == 0), stop=(j == NGT - 1))
                    return hps

                def norm_quad(q0, hps):
                    tp = HP.tile([128, 4, 128], F16, tag="hvtp", name="tp")
                    hns = []
                    for dq in range(4):
                        rinv = RN.tile([128, 1], F32, tag="rinv", name="rinv")
                        nc.vector.reciprocal_approx_fast(
                            rinv[:], hps[:, dq, DK:DK + 1])
                        hnorm = RN.tile([128, DK], F16, tag="hnorm",
                                        name="hnorm")
                        nc.vector.tensor_scalar(hnorm[:], hps[:, dq, 0:DK],
                                                rinv[:], None, MUL)
                        hns.append(hnorm)
                    for dq in range(4):
                        nc.tensor.transpose(tp[i * 64:(i + 1) * 64, dq, :],
                                            hns[dq][:], ident[:],
                                            tile_position=(0, i * 64))
                    for dq in range(4):
                        qc = q0 + dq
                        nc.vector.tensor_copy(
                            htn[i * 64:(i + 1) * 64, hp,
                                qc * 128:(qc + 1) * 128],
                            tp[i * 64:(i + 1) * 64, dq, :])

                def chunk0():
                    hvs[0] = hv_quad(0)
                def chunk1():
                    hvs[1] = hv_quad(4)
                def chunk2():
                    norm_quad(0, hvs[0])
                    if post_quad:
                        post_quad(0)
                def chunk3():
                    norm_quad(4, hvs[1])
                    if post_quad:
                        post_quad(1)
                hvs = [None, None]
                chunks = [chunk0, chunk1, chunk2, chunk3]
                if defer_hv:
                    return chunks
                for c in chunks:
                    c()

            pq0 = proj_qk_groups(0)
            for g in pq0[:4]:
                g()
            vb = proj_v_groups(1)
            prev = attn_head(0, interleave=pq0[4:] + proj_v_groups(0),
                             defer_hv=True)
            prev = attn_head(1, interleave=proj_qk_groups(1) + prev,
                             defer_hv=True)
            prev = attn_head(2, interleave=vb[:8] + prev, defer_hv=True)
            prev = attn_head(3, interleave=proj_qk_groups(2) + prev,
                             defer_hv=True)
            prev = attn_head(4, interleave=vb[8:] + prev, defer_hv=True)
            prev = attn_head(5, interleave=proj_qk_groups(3) + prev,
                             defer_hv=True)
            prev = attn_head(6, interleave=prev, defer_hv=True)
            with tc.tile_pool(name="ob", bufs=8) as OB:
                def out_proj(qts):
                    for qt in qts:
                        po = SP.tile([128, 512], F32, tag="s2", name="po")
                        for hp in range(NHP):
                            nc.tensor.matmul(
                                po[:],
                                htn[:, hp, qt * 128:(qt + 1) * 128],
                                wob[:, hp * 512:(hp + 1) * 512],
                                start=(hp == 0), stop=(hp == NHP - 1))
                        ob = OB.tile([128, 512], F32, tag="ob", name="ob")
                        nc.scalar.copy(ob[:], po[:])
                        nc.sync.dma_start(
                            out=out[qt * 128:(qt + 1) * 128, :], in_=ob[:])

                attn_head(7, interleave=prev,
                          post_quad=lambda half: out_proj(
                              range(half * 4, half * 4 + 4)))
            XT.release()


    nc.compile()
    _CACHE["nc"] = nc
    return nc


def kernel(q, mask, W_query, W_key, W_val, W_out):
    q = np.asarray(q, dtype=np.float32)
    mask = np.asarray(mask, dtype=np.int32)
    # [f, hp*128 + i*64 + dk] for the q/k projections (head-pair packed),
    # [f, h*64 + dv] for v, [i*64 + dk, hp*512 + e] for the output projection.
    wq_r = np.ascontiguousarray(np.transpose(
        np.asarray(W_query, np.float32), (1, 0, 2)).reshape(D, D)).astype(np.float16)
    wk_r = np.ascontiguousarray(np.transpose(
        np.asarray(W_key, np.float32), (1, 0, 2)).reshape(D, D)).astype(np.float16)
    wv_r = np.ascontiguousarray(np.transpose(
        np.asarray(W_val, np.float32), (1, 0, 2)).reshape(D, D)).astype(np.float16)
    wo_r = np.ascontiguousarray(
        np.asarray(W_out, np.float32).reshape(NHP, 2, DK, D)
        .transpose(1, 2, 0, 3).reshape(128, NHP * D)).astype(np.float16)

    nc = _build()
    in_maps = []
    for c in range(NCORES):
        b, qh = c // 2, c % 2
        xqt_c = q[b].T                                      # (D, N)
        nmt_c = 1.0 - mask[b, qh * NQ:(qh + 1) * NQ, :].T   # (N, NQ)
        if qh:
            # rotate the key axis so this core's queries are tokens [0, NQ)
            xqt_c = np.roll(xqt_c, -NQ, axis=1)
            nmt_c = np.roll(nmt_c, -NQ, axis=0)
        wqk0 = np.concatenate(
            [wr[:, 0:128].reshape(NFC, 128, 128).transpose(1, 0, 2)
             .reshape(128, NFC * 128) for wr in (wq_r, wk_r)],
            axis=1)
        imap = {
            "wqk0": np.ascontiguousarray(wqk0),
            "xqt": np.ascontiguousarray(xqt_c.astype(np.float16)),
            "nmt": np.ascontiguousarray(nmt_c.astype(np.float16)),
            "wq": wq_r, "wk": wk_r, "wv": wv_r, "wo": wo_r,
        }
        if MBGTS:
            mrows = np.concatenate([1.0 - nmt_c[g * 128:(g + 1) * 128, :]
                                    for g in MBGTS], axis=0)
            imap["mb"] = np.ascontiguousarray(
                np.float32(BETA)
                - np.float32(240.0 * ALPHA) * mrows.astype(np.float32))
        in_maps.append(imap)
    res = run_bass_kernel_spmd(nc, in_maps, core_ids=list(range(NCORES)))
    output = np.empty((B, N, D), np.float32)
    for c in range(NCORES):
        b, qh = c // 2, c % 2
        output[b, qh * NQ:(qh + 1) * NQ, :] = res.results[c]["out"]
    return output
